# revision 10
# baseline (speedup 1.0000x reference)
"""GAT (2-layer, single-head) Trainium2 Bass kernel, 8-core SPMD.

Strategy (edge/graph parallelism per the sharding hint):
  - Destination nodes are 1D-sharded: core c owns nodes [c*12500, (c+1)*12500).
  - Edges are routed to the core that owns their destination (host-side
    bucketing by dst), grouped into 128-node dst blocks.
  - Each core computes its slice of the per-node feature table
    feat_aug = [1 | x@W | x@W@al | x@W@ar]  ([N, 35]) and the full table is
    AllGathered so every core can gather arbitrary src rows.
  - Edge phase per 128-dst block: indirect-DMA gather of feat_aug[src] rows
    and er[dst] scalars, attention logits e = leaky_relu(el[src] + er[dst]),
    ex = exp(e) (softmax shift-invariance: max-subtraction dropped; logits
    are O(1) so fp32 exp is safe), and a one-hot matmul segment reduction:
    the one-hot of dst-local ids is built scaled by ex in a single two-op
    tensor_scalar (is_equal then mult), so out = onehot_ex^T @ [1 | feat]
    accumulates both the softmax denominator (col 0) and the numerator in
    one PSUM accumulation chain per block.
  - out_block = numerator / denom + bias (+ relu between layers); layer-2
    feature table is produced inline per block (PE transpose + matmul) and
    AllGathered; each core writes its own [12500, 32] output slice.
"""

import numpy as np

N = 100000
E = 1600000
F = 128
H = 32
NCORES = 8
NPC = N // NCORES          # nodes per core
P = 128
NB = (NPC + P - 1) // P    # dst blocks per core (98; last block 84 rows)
LB = NPC - (NB - 1) * P    # rows in last block
TW = 1 + H + 2             # table row: [1, feat(32), el, er]

_cache = {}


def _host_prep(x, src, dst, W1, al1, ar1, b1, W2, al2, ar2, b2):
    f32, i32 = np.float32, np.int32
    src = np.asarray(src).astype(np.int64)
    dst = np.asarray(dst).astype(np.int64)

    order = np.argsort(dst, kind="stable")
    s_src = src[order].astype(i32)
    s_dst = dst[order].astype(i32)

    c = s_dst // NPC
    r = s_dst % NPC
    b = r // P
    dl = (r % P).astype(f32)
    key = c * NB + b

    counts = np.bincount(key, minlength=NCORES * NB).reshape(NCORES, NB)
    K_b = np.maximum(1, -(-counts.max(axis=0) // P))        # chunks per block slot
    offs = np.concatenate([[0], np.cumsum(K_b)])            # chunk col offsets
    CH = int(offs[-1])

    # rank of each edge within its (core, block) segment
    seg_start = np.concatenate([[0], np.cumsum(counts.ravel())])[:-1]
    rank = np.arange(len(s_dst), dtype=np.int64) - seg_start[key]
    p = rank % P
    col = offs[b] + rank // P

    srcg = np.zeros((NCORES, P, CH), i32)
    dstgf = np.zeros((NCORES, P, CH), i32)
    dstl = np.full((NCORES, P, CH), 200.0, f32)
    flat = c * (P * CH) + p * CH + col
    srcg.reshape(-1)[flat] = s_src
    dstgf.reshape(-1)[flat] = s_dst * TW + (TW - 1)
    dstl.reshape(-1)[flat] = dl

    def aug(W, al, ar):
        Wa = np.zeros((W.shape[0], TW), f32)
        Wa[:, 1:1 + H] = W
        Wa[:, 1 + H] = W @ al
        Wa[:, 2 + H] = W @ ar
        return Wa

    W1a = aug(np.asarray(W1, f32), np.asarray(al1, f32), np.asarray(ar1, f32))
    W2a = aug(np.asarray(W2, f32), np.asarray(al2, f32), np.asarray(ar2, f32))
    b1r = np.tile(np.asarray(b1, f32)[None, :], (P, 1))
    b2r = np.tile(np.asarray(b2, f32)[None, :], (P, 1))
    iota = np.tile(np.arange(P, dtype=f32)[None, :], (P, 1))

    x = np.asarray(x, f32)
    xsT = [np.ascontiguousarray(x[cc * NPC:(cc + 1) * NPC].T) for cc in range(NCORES)]

    in_maps = []
    for cc in range(NCORES):
        in_maps.append({
            "xsT": xsT[cc],
            "W1a": W1a, "W2a": W2a, "b1r": b1r, "b2r": b2r, "iota": iota,
            "srcg": srcg[cc], "dstgf": dstgf[cc], "dstl": dstl[cc],
        })
    return in_maps, tuple(int(k) for k in K_b)


def _build_program(K_list, debug=()):
    import concourse.bacc as bacc
    import concourse.mybir as mybir
    import concourse.tile as tile
    from concourse import bass
    from concourse.bass import IndirectOffsetOnAxis
    from concourse.masks import make_identity

    dt = mybir.dt
    K_list = list(K_list)
    CH = sum(K_list)
    offs = np.concatenate([[0], np.cumsum(K_list)]).astype(int)

    nc = bacc.Bacc("TRN2", target_bir_lowering=False, debug=False,
                   num_devices=NCORES)

    xsT = nc.dram_tensor("xsT", [F, NPC], dt.float32, kind="ExternalInput")
    W1a = nc.dram_tensor("W1a", [F, TW], dt.float32, kind="ExternalInput")
    W2a = nc.dram_tensor("W2a", [H, TW], dt.float32, kind="ExternalInput")
    b1r = nc.dram_tensor("b1r", [P, H], dt.float32, kind="ExternalInput")
    b2r = nc.dram_tensor("b2r", [P, H], dt.float32, kind="ExternalInput")
    iota = nc.dram_tensor("iota", [P, P], dt.float32, kind="ExternalInput")
    srcg = nc.dram_tensor("srcg", [P, CH], dt.int32, kind="ExternalInput")
    dstgf = nc.dram_tensor("dstgf", [P, CH], dt.int32, kind="ExternalInput")
    dstl = nc.dram_tensor("dstl", [P, CH], dt.float32, kind="ExternalInput")
    out_ext = nc.dram_tensor("out", [NPC, H], dt.float32, kind="ExternalOutput")
    dbg_ext = {}
    if "feat1" in debug:
        dbg_ext["feat1"] = nc.dram_tensor("dbg_feat1", [N, TW], dt.float32,
                                          kind="ExternalOutput")
    if "edge1" in debug:
        dbg_ext["gath"] = nc.dram_tensor("dbg_gath", [P, sum(K_list) * TW],
                                         dt.float32, kind="ExternalOutput")
        dbg_ext["erg"] = nc.dram_tensor("dbg_erg", [P, sum(K_list)],
                                        dt.float32, kind="ExternalOutput")
        dbg_ext["ex"] = nc.dram_tensor("dbg_ex", [P, sum(K_list)],
                                       dt.float32, kind="ExternalOutput")
        dbg_ext["pacc"] = nc.dram_tensor("dbg_pacc", [NB * P, 1 + H],
                                         dt.float32, kind="ExternalOutput")

    with tile.TileContext(nc) as tc:
        with (
            tc.tile_pool(name="const", bufs=1) as const,
            tc.tile_pool(name="prod", bufs=3) as prod,
            tc.tile_pool(name="gath", bufs=3) as gpool,
            tc.tile_pool(name="edge", bufs=4) as epool,
            tc.tile_pool(name="oh", bufs=6) as ohpool,
            tc.tile_pool(name="epi", bufs=3) as epipool,
            tc.tile_pool(name="ps", bufs=3, space="PSUM") as psum,
            tc.tile_pool(name="pst", bufs=2, space="PSUM") as psumt,
            tc.tile_pool(name="dram", bufs=1, space="DRAM") as dram,
        ):
            # ---- constants into SBUF ----
            iota_sb = const.tile([P, P], dt.float32)
            nc.sync.dma_start(out=iota_sb[:], in_=iota[:])
            W1a_sb = const.tile([F, TW], dt.float32)
            nc.sync.dma_start(out=W1a_sb[:], in_=W1a[:])
            W2a_sb = const.tile([H, TW], dt.float32)
            nc.sync.dma_start(out=W2a_sb[:], in_=W2a[:])
            b1r_sb = const.tile([P, H], dt.float32)
            nc.sync.dma_start(out=b1r_sb[:], in_=b1r[:])
            b2r_sb = const.tile([P, H], dt.float32)
            nc.sync.dma_start(out=b2r_sb[:], in_=b2r[:])
            srcg_sb = const.tile([P, CH], dt.int32)
            nc.sync.dma_start(out=srcg_sb[:], in_=srcg[:])
            dstgf_sb = const.tile([P, CH], dt.int32)
            nc.sync.dma_start(out=dstgf_sb[:], in_=dstgf[:])
            dstl_sb = const.tile([P, CH], dt.float32)
            nc.sync.dma_start(out=dstl_sb[:], in_=dstl[:])
            ident = const.tile([P, P], dt.float32)
            make_identity(nc, ident[:])

            feat1_s = dram.tile([NPC, TW], dt.float32)
            feat1_f = dram.tile([N, TW], dt.float32, addr_space="Shared")
            feat2_s = dram.tile([NPC, TW], dt.float32)
            feat2_f = dram.tile([N, TW], dt.float32, addr_space="Shared")

            # ---- layer-1 feature table production: feat1 = [1 | x@W1aug] ----
            for b in range(NB):
                rows = LB if b == NB - 1 else P
                xt = prod.tile([F, P], dt.float32, tag="xt")
                nc.sync.dma_start(out=xt[:, :rows],
                                  in_=xsT[:, b * P: b * P + rows])
                pmm = psumt.tile([P, TW], dt.float32, tag="pmm")
                nc.tensor.matmul(out=pmm[:rows, :], lhsT=xt[:, :rows],
                                 rhs=W1a_sb[:], start=True, stop=True)
                fsb = prod.tile([P, TW], dt.float32, tag="fsb")
                nc.vector.tensor_copy(out=fsb[:rows, 1:], in_=pmm[:rows, 1:])
                nc.vector.memset(fsb[:rows, 0:1], 1.0)
                nc.sync.dma_start(out=feat1_s[b * P: b * P + rows, :],
                                  in_=fsb[:rows, :])

            nc.gpsimd.collective_compute(
                "AllGather", mybir.AluOpType.bypass,
                replica_groups=[list(range(NCORES))],
                ins=[feat1_s[:]], outs=[feat1_f[:]],
            )

            if "feat1" in dbg_ext:
                nc.sync.dma_start(out=dbg_ext["feat1"][:], in_=feat1_f[:])

            # ---- edge phase (shared for both layers) ----
            def edge_phase(feat_f, bias_sb, relu, out_writer, dbg=False):
                feat_flat = feat_f[:].rearrange("n d -> (n d)")[:, None]
                for b in range(NB):
                    K = K_list[b]
                    O = int(offs[b])
                    rows = LB if b == NB - 1 else P
                    gath = gpool.tile([P, K * TW], dt.float32, tag="gath")
                    erg = epool.tile([P, K], dt.float32, tag="erg")
                    for k in range(K):
                        nc.gpsimd.indirect_dma_start(
                            out=gath[:, k * TW:(k + 1) * TW], out_offset=None,
                            in_=feat_f[:],
                            in_offset=IndirectOffsetOnAxis(
                                ap=srcg_sb[:, O + k:O + k + 1], axis=0),
                        )
                        nc.gpsimd.indirect_dma_start(
                            out=erg[:, k:k + 1], out_offset=None,
                            in_=feat_flat,
                            in_offset=IndirectOffsetOnAxis(
                                ap=dstgf_sb[:, O + k:O + k + 1], axis=0),
                        )
                    # e = leaky_relu(el + er); ex = exp(e)
                    el = gath[:].rearrange("p (k w) -> p k w", w=TW)[:, :, TW - 2]
                    ee = epool.tile([P, K], dt.float32, tag="ee")
                    nc.vector.tensor_tensor(out=ee[:], in0=el, in1=erg[:],
                                            op=mybir.AluOpType.add)
                    et = epool.tile([P, K], dt.float32, tag="et")
                    nc.vector.tensor_scalar_mul(out=et[:], in0=ee[:],
                                                scalar1=0.2)
                    nc.vector.tensor_tensor(out=ee[:], in0=ee[:], in1=et[:],
                                            op=mybir.AluOpType.max)
                    ex = epool.tile([P, K], dt.float32, tag="ex")
                    nc.scalar.activation(out=ex[:], in_=ee[:],
                                         func=mybir.ActivationFunctionType.Exp)
                    if dbg and "gath" in dbg_ext:
                        nc.sync.dma_start(
                            out=dbg_ext["gath"][:, O * TW:(O + K) * TW],
                            in_=gath[:])
                        nc.sync.dma_start(out=dbg_ext["erg"][:, O:O + K],
                                          in_=erg[:])
                        nc.sync.dma_start(out=dbg_ext["ex"][:, O:O + K],
                                          in_=ex[:])
                    pacc = psum.tile([P, 1 + H], dt.float32, tag="pacc")
                    for k in range(K):
                        oh = ohpool.tile([P, P], dt.float32, tag="oh")
                        nc.vector.tensor_scalar(
                            out=oh[:], in0=iota_sb[:],
                            scalar1=dstl_sb[:, O + k: O + k + 1],
                            scalar2=ex[:, k: k + 1],
                            op0=mybir.AluOpType.is_equal,
                            op1=mybir.AluOpType.mult,
                        )
                        nc.tensor.matmul(
                            out=pacc[:], lhsT=oh[:],
                            rhs=gath[:, k * TW: k * TW + 1 + H],
                            start=(k == 0), stop=(k == K - 1),
                        )
                    if dbg and "pacc" in dbg_ext:
                        pc = epipool.tile([P, 1 + H], dt.float32, tag="pc")
                        nc.vector.tensor_copy(out=pc[:], in_=pacc[:])
                        nc.sync.dma_start(
                            out=dbg_ext["pacc"][b * P:(b + 1) * P, :],
                            in_=pc[:])
                    # epilogue: h = numer / denom + bias (+ relu)
                    den = epipool.tile([P, 1], dt.float32, tag="den")
                    nc.vector.tensor_scalar_add(out=den[:], in0=pacc[:, 0:1],
                                                scalar1=1e-30)
                    rec = epipool.tile([P, 1], dt.float32, tag="rec")
                    nc.vector.reciprocal(out=rec[:], in_=den[:])
                    h = epipool.tile([P, H], dt.float32, tag="h")
                    nc.vector.tensor_scalar_mul(out=h[:rows, :],
                                                in0=pacc[:rows, 1:],
                                                scalar1=rec[:rows, :])
                    nc.vector.tensor_tensor(out=h[:rows, :], in0=h[:rows, :],
                                            in1=bias_sb[:rows, :],
                                            op=mybir.AluOpType.add)
                    if relu:
                        nc.scalar.activation(
                            out=h[:rows, :], in_=h[:rows, :],
                            func=mybir.ActivationFunctionType.Relu)
                    out_writer(b, rows, h)

            # layer-1: write h into feat2 table production
            def l1_writer(b, rows, h):
                pt = psumt.tile([H, P], dt.float32, tag="pt")
                nc.tensor.transpose(out=pt[:, :], in_=h[:, :], identity=ident[:])
                hT = prod.tile([H, P], dt.float32, tag="hT")
                nc.vector.tensor_copy(out=hT[:], in_=pt[:])
                pmm2 = psumt.tile([P, TW], dt.float32, tag="pmm")
                nc.tensor.matmul(out=pmm2[:rows, :], lhsT=hT[:, :rows],
                                 rhs=W2a_sb[:], start=True, stop=True)
                f2 = prod.tile([P, TW], dt.float32, tag="f2")
                nc.vector.tensor_copy(out=f2[:rows, 1:], in_=pmm2[:rows, 1:])
                nc.vector.memset(f2[:rows, 0:1], 1.0)
                nc.sync.dma_start(out=feat2_s[b * P: b * P + rows, :],
                                  in_=f2[:rows, :])

            edge_phase(feat1_f, b1r_sb, True, l1_writer, dbg=True)

            nc.gpsimd.collective_compute(
                "AllGather", mybir.AluOpType.bypass,
                replica_groups=[list(range(NCORES))],
                ins=[feat2_s[:]], outs=[feat2_f[:]],
            )

            # layer-2: write final output slice
            def l2_writer(b, rows, h):
                nc.sync.dma_start(out=out_ext[b * P: b * P + rows, :],
                                  in_=h[:rows, :])

            edge_phase(feat2_f, b2r_sb, False, l2_writer)

    nc.compile()
    return nc


def _get_program(K_list):
    key = ("prog", K_list)
    if key not in _cache:
        _cache[key] = _build_program(K_list)
    return _cache[key]


def kernel(x, src, dst, W1, al1, ar1, b1, W2, al2, ar2, b2):
    from concourse.bass_utils import run_bass_kernel_spmd

    in_maps, K_list = _host_prep(x, src, dst, W1, al1, ar1, b1,
                                 W2, al2, ar2, b2)
    nc = _get_program(K_list)
    res = run_bass_kernel_spmd(nc, in_maps, list(range(NCORES)))
    out = np.concatenate([res.results[c]["out"] for c in range(NCORES)], axis=0)
    return out.astype(np.float32)
